# revision 5
# baseline (speedup 1.0000x reference)
"""Capsule-network dynamic-routing kernel for 8x TRN2 NeuronCores.

Math (reference):
    u_hat[b,c,u,o] = sum_i W[c,u,o,i] * x[b,i,c]          (never materialized!)
    repeat 3x:
        cw = softmax(b_logits, axis=u)                    # [C, U]
        s[b,u,o]  = sum_c cw[c,u] * u_hat[b,c,u,o]
        v = squash(s)
        agree[c,u] = mean_b sum_o u_hat[b,c,u,o]*v[b,u,o]
        b_logits += agree
    return v[..., None]

Key identities used (u_hat is 377 MB, so we fuse it away):
    s[b,(u,o)]   = sum_{(i,c)} x[b,i,c] * (cw[c,u]*W[c,u,o,i])
                 -> one K=9216 matmul per iteration, rhs = cw-scaled W.
    agree[c,u]   = (1/B) sum_{o,i} W[c,u,o,i] * P[(i,c),(u,o)]
      where P[(i,c),(u,o)] = sum_b x[b,i,c]*v[b,u,o]  (rank-64 matmul)
                 -> DVE multiply-reduce against the same W layout.

Sharding: batch 512 -> 64 per core (pure data parallel); W replicated.
The only cross-core communication is an AllReduce of the [1152,10] agree
partials after routing iterations 1 and 2 (iteration 3's agree is unused).

Per-core SBUF layouts, chunk index t = j*8 + i (j: c-chunk of 128, i: in_unit):
    x_sb [64,  72, 128] : x[b, i, 128j+q]          (lhsT of P-matmul)
    xT_sb[128, 72, 64]  : x transposed per chunk   (lhsT of s-matmul)
    Wt_sb[128, 72, 160] : W[c, u, o, i] as [(c), t, (u,o)]  (rhs / agree operand)
"""

import sys

sys.path.insert(0, "/opt/trn_rl_repo")

import numpy as np

import concourse.bacc as bacc
import concourse.tile as tile
from concourse import mybir
from concourse.masks import make_identity

F32 = mybir.dt.float32
MULT = mybir.AluOpType.mult
ADD = mybir.AluOpType.add
AF = mybir.ActivationFunctionType
AX = mybir.AxisListType

N_CORES = 8
B_FULL = 512
BSH = B_FULL // N_CORES  # 64 batch rows per core
C = 1152                 # in_channel
U = 10                   # num_unit
O = 16                   # unit_size
I8 = 8                   # in_unit
J = C // 128             # 9 chunks of 128 channels
NT = J * I8              # 72 (i,c)-chunks of 128 contraction rows
UO = U * O               # 160
NR = 3                   # routing iterations


def build_nc(n_cores=N_CORES, collective=True):
    nc = bacc.Bacc("TRN2", target_bir_lowering=False, debug=False,
                   num_devices=n_cores)
    xs = nc.dram_tensor("xs", [BSH, I8, C], F32, kind="ExternalInput")
    wd = nc.dram_tensor("w", [C, U, O, I8], F32, kind="ExternalInput")
    vout = nc.dram_tensor("v", [BSH, UO], F32, kind="ExternalOutput")

    with tile.TileContext(nc) as tc:
        with (
            tc.tile_pool(name="persist", bufs=1) as per,
            tc.tile_pool(name="stage", bufs=2) as stage,
            tc.tile_pool(name="small", bufs=2) as small,
            tc.tile_pool(name="wpj", bufs=3) as wpj_pool,
            tc.tile_pool(name="ppj", bufs=3) as ppj_pool,
            tc.tile_pool(name="dram", bufs=2, space="DRAM") as dram,
        ):
            x_sb = per.tile([BSH, NT, 128], F32)
            xT_sb = per.tile([128, NT, BSH], F32)
            wt_sb = per.tile([128, NT, UO], F32)
            cw_sb = per.tile([128, J, U], F32)
            b_sb = per.tile([128, J, U], F32)
            v_sb = per.tile([BSH, U, O], F32)
            ident = per.tile([128, 128], F32)

            make_identity(nc, ident[:])
            nc.vector.memset(b_sb[:], 0.0)

            # ---- load x, build per-chunk transpose xT ----
            with tc.tile_pool(name="ptr", bufs=4, space="PSUM") as ptr:
                for j in range(J):
                    nc.gpsimd.dma_start(
                        out=x_sb[:, j * I8:(j + 1) * I8, :],
                        in_=xs[:, :, j * 128:(j + 1) * 128],
                    )
                for t in range(NT):
                    trp = ptr.tile([128, BSH], F32, tag="trp")
                    nc.tensor.transpose(
                        out=trp[:], in_=x_sb[:, t, :],
                        identity=ident[:BSH, :BSH],
                    )
                    nc.scalar.activation(out=xT_sb[:, t, :], in_=trp[:],
                                         func=AF.Copy)
                # ---- load W, rearrange to [(c), t, (u,o)] ----
                for j in range(J):
                    wn = stage.tile([128, U, O, I8], F32, tag="wn")
                    nc.gpsimd.dma_start(
                        out=wn[:], in_=wd[j * 128:(j + 1) * 128, :, :, :])
                    for i in range(I8):
                        t = j * I8 + i
                        dst = wt_sb[:, t, :].rearrange("p (u o) -> p u o", u=U)
                        eng = nc.vector if i % 2 == 0 else nc.gpsimd
                        eng.tensor_copy(out=dst, in_=wn[:, :, :, i])

            # ---- routing iterations ----
            with (
                tc.tile_pool(name="ps", bufs=2, space="PSUM") as ps_pool,
                tc.tile_pool(name="ppp", bufs=4, space="PSUM") as ppp_pool,
            ):
                for r in range(NR):
                    # s = x @ (cw*W); iteration 0 has cw = 0.1 uniform, so use
                    # unscaled Wt and fold alpha=0.1 into the squash.
                    alpha = 0.1 if r == 0 else 1.0
                    s_ps = ps_pool.tile([BSH, UO], F32, tag="s")
                    if r == 0:
                        for t in range(NT):
                            nc.tensor.matmul(
                                s_ps[:], xT_sb[:, t, :], wt_sb[:, t, :],
                                start=(t == 0), stop=(t == NT - 1),
                            )
                    else:
                        for j in range(J):
                            wpj = wpj_pool.tile([128, I8, UO], F32, tag="wpj")
                            for u in range(U):
                                nc.vector.tensor_scalar_mul(
                                    out=wpj[:, :, u * O:(u + 1) * O],
                                    in0=wt_sb[:, j * I8:(j + 1) * I8,
                                              u * O:(u + 1) * O],
                                    scalar1=cw_sb[:, j, u:u + 1],
                                )
                            for i in range(I8):
                                t = j * I8 + i
                                nc.tensor.matmul(
                                    s_ps[:], xT_sb[:, t, :], wpj[:, i, :],
                                    start=(t == 0), stop=(t == NT - 1),
                                )

                    # squash: n2 = sum_o (alpha*s)^2 ; v = alpha*s*sqrt(n2)/(1+n2)
                    n2 = small.tile([BSH, U], F32, tag="n2")
                    sq = small.tile([BSH, O], F32, tag="sq")
                    for u in range(U):
                        nc.scalar.activation(
                            out=sq[:],
                            in_=s_ps[:, u * O:(u + 1) * O],
                            func=AF.Square, scale=alpha,
                            accum_out=n2[:, u:u + 1],
                        )
                    rt = small.tile([BSH, U], F32, tag="rt")
                    nc.scalar.activation(out=rt[:], in_=n2[:], func=AF.Sqrt)
                    dn = small.tile([BSH, U], F32, tag="dn")
                    nc.vector.tensor_scalar_add(out=dn[:], in0=n2[:],
                                                scalar1=1.0)
                    rec = small.tile([BSH, U], F32, tag="rec")
                    nc.vector.reciprocal(out=rec[:], in_=dn[:])
                    gf = small.tile([BSH, U], F32, tag="gf")
                    nc.vector.tensor_mul(gf[:], rt[:], rec[:])
                    for u in range(U):
                        nc.vector.tensor_scalar(
                            out=v_sb[:, u, :],
                            in0=s_ps[:, u * O:(u + 1) * O],
                            scalar1=gf[:, u:u + 1], scalar2=alpha,
                            op0=MULT, op1=MULT,
                        )

                    if r == NR - 1:
                        nc.gpsimd.dma_start(
                            out=vout[:, :],
                            in_=v_sb[:].rearrange("b u o -> b (u o)"),
                        )
                        break

                    # agree[c,u] = (1/B) sum_{o,i} W*(x^T @ v), then AllReduce.
                    # 1/B is folded into the PSUM eviction (activation scale).
                    agree_sb = small.tile([128, J, U], F32, tag="agree")
                    vflat = v_sb[:].rearrange("b u o -> b (u o)")
                    for j in range(J):
                        ppj = ppj_pool.tile([128, I8, UO], F32, tag="ppj")
                        for i in range(I8):
                            t = j * I8 + i
                            pp_ps = ppp_pool.tile([128, UO], F32, tag="pp")
                            nc.tensor.matmul(pp_ps[:], x_sb[:, t, :], vflat,
                                             start=True, stop=True)
                            nc.scalar.activation(out=ppj[:, i, :],
                                                 in_=pp_ps[:], func=AF.Copy,
                                                 scale=1.0 / B_FULL)
                        qj = ppj_pool.tile([128, I8, UO], F32, tag="qj")
                        nc.vector.tensor_mul(
                            qj[:], wt_sb[:, j * I8:(j + 1) * I8, :], ppj[:])
                        nc.vector.reduce_sum(
                            out=agree_sb[:, j, :],
                            in_=qj[:].rearrange("p i (u o) -> p u i o", u=U),
                            axis=AX.XY)

                    cin = dram.tile([128, J * U], F32, tag="cin")
                    cout = dram.tile([128, J * U], F32, tag="cout")
                    nc.gpsimd.dma_start(
                        out=cin[:],
                        in_=agree_sb[:].rearrange("p j u -> p (j u)"))
                    if collective:
                        nc.gpsimd.collective_compute(
                            "AllReduce", ADD,
                            replica_groups=[list(range(n_cores))],
                            ins=[cin.opt()], outs=[cout.opt()],
                        )
                    else:
                        nc.gpsimd.dma_start(out=cout[:], in_=cin[:])
                    ar = small.tile([128, J, U], F32, tag="ar")
                    nc.gpsimd.dma_start(
                        out=ar[:].rearrange("p j u -> p (j u)"), in_=cout[:])
                    nc.vector.tensor_add(b_sb[:], b_sb[:], ar[:])

                    # cw = softmax(b, axis=u), numerically stabilized
                    for j in range(J):
                        nmx = small.tile([128, 1], F32, tag="nmx")
                        nc.vector.reduce_max(out=nmx[:], in_=b_sb[:, j, :],
                                             axis=AX.X, negate=True)
                        et = small.tile([128, U], F32, tag="et")
                        se = small.tile([128, 1], F32, tag="se")
                        nc.scalar.activation(out=et[:], in_=b_sb[:, j, :],
                                             func=AF.Exp, bias=nmx[:],
                                             scale=1.0, accum_out=se[:])
                        rse = small.tile([128, 1], F32, tag="rse")
                        nc.vector.reciprocal(out=rse[:], in_=se[:])
                        nc.vector.tensor_scalar_mul(out=cw_sb[:, j, :],
                                                    in0=et[:], scalar1=rse[:])

    nc.finalize()
    return nc


class _Runner:
    """Compile once, run many times (jax.jit cache kept alive)."""

    def __init__(self):
        import jax
        from jax.sharding import Mesh, PartitionSpec
        from jax.experimental.shard_map import shard_map
        from concourse import bass2jax

        bass2jax.install_neuronx_cc_hook()
        nc = build_nc()
        self.nc = nc

        in_names, out_names, out_avals = [], [], []
        partition_name = (nc.partition_id_tensor.name
                          if nc.partition_id_tensor else None)
        for alloc in nc.m.functions[0].allocations:
            if not isinstance(alloc, mybir.MemoryLocationSet):
                continue
            name = alloc.memorylocations[0].name
            if alloc.kind == "ExternalInput":
                if name != partition_name:
                    in_names.append(name)
            elif alloc.kind == "ExternalOutput":
                out_names.append(name)
                out_avals.append(jax.core.ShapedArray(
                    tuple(alloc.tensor_shape), mybir.dt.np(alloc.dtype)))
        self.in_names = list(in_names)
        self.out_names = out_names
        self.out_avals = out_avals
        n_params = len(in_names)
        n_outs = len(out_names)
        all_names = in_names + out_names
        if partition_name is not None:
            all_names = all_names + [partition_name]

        def _body(*args):
            operands = list(args)
            if partition_name is not None:
                operands.append(bass2jax.partition_id_tensor())
            outs = bass2jax._bass_exec_p.bind(
                *operands,
                out_avals=tuple(out_avals),
                in_names=tuple(all_names),
                out_names=tuple(out_names),
                lowering_input_output_aliases=(),
                sim_require_finite=True,
                sim_require_nnan=True,
                nc=nc,
            )
            return tuple(outs)

        devices = jax.devices()[:N_CORES]
        mesh = Mesh(np.asarray(devices), ("core",))
        in_specs = (PartitionSpec("core"),) * (n_params + n_outs)
        out_specs = (PartitionSpec("core"),) * n_outs
        donate = tuple(range(n_params, n_params + n_outs))
        self._fn = jax.jit(
            shard_map(_body, mesh=mesh, in_specs=in_specs,
                      out_specs=out_specs, check_rep=False),
            donate_argnums=donate, keep_unused=True,
        )

    def run(self, in_maps):
        concat_in = [
            np.concatenate([np.asarray(in_maps[c][n]) for c in range(N_CORES)],
                           axis=0)
            for n in self.in_names
        ]
        zeros = [
            np.zeros((N_CORES * a.shape[0],) + tuple(a.shape[1:]), a.dtype)
            for a in self.out_avals
        ]
        outs = self._fn(*concat_in, *zeros)
        return [
            {
                n: np.asarray(outs[i]).reshape(
                    (N_CORES,) + tuple(self.out_avals[i].shape))[c]
                for i, n in enumerate(self.out_names)
            }
            for c in range(N_CORES)
        ]


_RUNNER = None


def _get_runner():
    global _RUNNER
    if _RUNNER is None:
        _RUNNER = _Runner()
    return _RUNNER


def kernel(x, W):
    x = np.ascontiguousarray(np.asarray(x, dtype=np.float32))
    W = np.ascontiguousarray(np.asarray(W, dtype=np.float32))
    assert x.shape == (B_FULL, I8, C), x.shape
    assert W.shape == (1, C, U, O, I8), W.shape
    w0 = np.ascontiguousarray(W[0])
    in_maps = [
        {"xs": np.ascontiguousarray(x[c * BSH:(c + 1) * BSH]), "w": w0}
        for c in range(N_CORES)
    ]
    results = _get_runner().run(in_maps)
    v = np.concatenate(
        [results[c]["v"].reshape(BSH, U, O) for c in range(N_CORES)], axis=0)
    return v[..., None].astype(np.float32)
